# revision 22
# baseline (speedup 1.0000x reference)
"""CenterLoss on 8 Trainium2 NeuronCores.

Math: the reference builds the full (B, C) squared-distance matrix,
masks it to the one entry (i, labels[i]) per row, clamps AFTER masking
(so the C-1 masked zeros per row each become 1e-12), sums and divides
by B.  Only the gathered center rows matter:

    loss = (sum_i clip(||x_i - c_{l_i}||^2, 1e-12, 1e12)
            + B*(C-1)*1e-12) / B

Sharding: data-parallel over the batch — core k gets rows
[k*256, (k+1)*256) of x/labels and a full replica of centers in DRAM.
Each core gathers its 256 needed center rows with two indirect DMAs
(reads 128 KB instead of 51 MB), computes per-row squared distances on
the vector engine, and writes the 256 distances out.  The host sums
the 8x256 partials and applies the constant clamp correction.
"""

import os

import numpy as np

BATCH = 2048
NUM_CLASSES = 100000
FEAT_DIM = 128
N_CORES = 8
ROWS_PER_CORE = BATCH // N_CORES  # 256
P = 128
TILES_PER_CORE = ROWS_PER_CORE // P  # 2

_CACHE = {}


def _build_v2(hoist=True, warmup=True, dram_off=False, hoist_pl=False,
              single_gather=False, split_out=False, pregather_nop=False,
              x_on_act=False, lab_single_packet=False, x_wait_lab=False):
    """Hand-synchronized raw-Bass kernel, v2.

    Differences vs the original raw kernel:
      - No entry pseudo-barrier: every semaphore is cleared on the same
        engine that first increments it, before that increment, so
        clear-vs-inc is ordered by the engine's program order.
      - DVE chain is 2 ops per tile (subtract, then scalar_tensor_tensor
        with op0=bypass/op1=mult and accum_out = row-sum of squares)
        instead of 3 whole-width ops, and is pipelined per gather: tile
        0's math runs while tile 1's gather is still in flight (separate
        completion semaphores per gather, since queue completion order
        is not guaranteed).
      - hoist: the labels/x DMAs (and their sem clears) are moved to the
        very front of the SP engine's stream, ahead of the framework
        preamble (reg-init movs, const-pool memsets, all-engine
        barrier), so the label round-trip overlaps the preamble.
      - warmup: a junk 8-row indirect gather is issued on gpsimd before
        the wait on labels, absorbing any one-time SWDGE setup cost
        while the engine would be idle anyway.
      - dram_off: the gathers' offset tables are the labels DRAM tensor
        directly (no labels DMA at all).  Experimental — depends on the
        SWDGE ucode accepting a DRAM offset AP.
    """
    from contextlib import ExitStack

    import concourse.bass as bass
    import concourse.mybir as mybir

    f32 = mybir.dt.float32
    i32 = mybir.dt.int32
    NT = TILES_PER_CORE
    D = FEAT_DIM

    # Row i of this core's shard maps to (partition, tile) = (i // NT,
    # i % NT): with row-index = p*NT + n every DMA's innermost dim is
    # contiguous in DRAM (tile-major row = n*P + p would stride it).
    nc = bass.Bass()
    x_d = nc.dram_tensor("x", [ROWS_PER_CORE, D], f32, kind="ExternalInput")
    lab_d = nc.dram_tensor("labels", [ROWS_PER_CORE, 1], i32, kind="ExternalInput")
    cen_d = nc.dram_tensor("centers", [NUM_CLASSES, D], f32, kind="ExternalInput")
    # split_out: dists[n, p] = distance of shard row p*NT + n (host
    # transposes back) — tile-major keeps each tile's store contiguous in
    # DRAM.  Combined: row-ordered [ROWS, 1] as one strided store.
    if split_out:
        out_d = nc.dram_tensor(
            "dists", [TILES_PER_CORE, P], f32, kind="ExternalOutput"
        )
    else:
        out_d = nc.dram_tensor(
            "dists", [ROWS_PER_CORE, 1], f32, kind="ExternalOutput"
        )

    hoisted = []  # mybir instructions to move to the front of the block

    with ExitStack() as ctx:
        x_all = ctx.enter_context(nc.sbuf_tensor([P, NT * D], f32))
        c_all = ctx.enter_context(nc.sbuf_tensor([P, NT * D], f32))
        dif = ctx.enter_context(nc.sbuf_tensor([P, NT * D], f32))
        sq = ctx.enter_context(nc.sbuf_tensor([P, NT * D], f32))
        s_all = ctx.enter_context(nc.sbuf_tensor([P, NT], f32))
        s_x = ctx.enter_context(nc.semaphore("s_x"))
        s_g = [
            ctx.enter_context(nc.semaphore(f"s_g{t}")) for t in range(NT)
        ]
        s_v = ctx.enter_context(nc.semaphore("s_v"))
        s_out = ctx.enter_context(nc.semaphore("s_out"))
        s_d = ctx.enter_context(nc.semaphore("s_d"))
        if not dram_off:
            idx = ctx.enter_context(nc.sbuf_tensor([P, NT], i32))
            s_lab = ctx.enter_context(nc.semaphore("s_lab"))
        if warmup:
            wtab = ctx.enter_context(nc.sbuf_tensor([P, 1], i32))
            wjunk = ctx.enter_context(nc.sbuf_tensor([P, D], f32))
            s_w = ctx.enter_context(nc.semaphore("s_w"))

        # --- SP (+ optionally ACT): input DMAs, hoisted to the stream
        # front if requested --------------------------------------------
        x_eng = nc.scalar if x_on_act else nc.sync
        if not dram_off:
            hoisted.append(nc.sync.sem_clear(s_lab).ins)
        hoisted.append(x_eng.sem_clear(s_x).ins)
        if not dram_off:
            hoisted.append(
                nc.sync.dma_start(
                    out=idx[:],
                    in_=lab_d[:].rearrange("(p n) o -> p (n o)", n=NT),
                    single_packet=lab_single_packet,
                )
                .then_inc(s_lab, 16)
                .ins
            )
        if x_wait_lab and not dram_off:
            # Keep the 1 KB labels transfer alone on the SDMA engines: the
            # 128 KB x load otherwise races it and adds ~0.6 us to the
            # labels completion half the time.  x is only needed by the
            # DVE at the first gather's completion (~3.5 us later), so
            # gating it on s_lab costs nothing.  NOT hoisted: a hoisted
            # pre-preamble wait would stall this engine's entry-barrier
            # participation and push the Pool engine's gathers back.
            x_eng.wait_ge(s_lab, 16)
            x_eng.dma_start(
                out=x_all[:].rearrange("p (n d) -> p n d", n=NT),
                in_=x_d[:].rearrange("(p n) d -> p n d", n=NT),
            ).then_inc(s_x, 16)
        else:
            hoisted.append(
                x_eng.dma_start(
                    out=x_all[:].rearrange("p (n d) -> p n d", n=NT),
                    in_=x_d[:].rearrange("(p n) d -> p n d", n=NT),
                )
                .then_inc(s_x, 16)
                .ins
            )

        # --- gpsimd: warmup, then the real gather(s) ------------------
        pl_insts = []  # collected for optional pre-preamble hoisting

        def pl(inst):
            pl_insts.append(inst.ins)
            return inst

        for s in s_g:
            pl(nc.gpsimd.sem_clear(s))
        if warmup:
            pl(nc.gpsimd.sem_clear(s_w))
            pl(nc.gpsimd.memset(wtab[:8, :], 0))
            pl(
                nc.gpsimd.indirect_dma_start(
                    out=wjunk[:8, :],
                    out_offset=None,
                    in_=cen_d[:],
                    in_offset=bass.IndirectOffsetOnAxis(ap=wtab[:8, :], axis=0),
                ).then_inc(s_w, 16)
            )
        if not dram_off:
            pl(nc.gpsimd.wait_ge(s_lab, 16))
        if pregather_nop:
            # Probe: does a cheap engine op between the wait and the first
            # indirect DMA absorb the ~0.9us post-wait latency?
            pl(nc.gpsimd.memset(wjunk[:1, :1] if warmup else c_all[:1, :1], 0))
        if single_gather:
            # One 256-descriptor gather: offsets idx[:, 0:2], destination
            # c_all[p, n*D:(n+1)*D] = centers[idx[p, n]].
            if dram_off:
                off_ap = lab_d[:].rearrange("(p n) o -> p (n o)", n=NT)
            else:
                off_ap = idx[:]
            pl(
                nc.gpsimd.indirect_dma_start(
                    out=c_all[:].rearrange("p (n d) -> p n d", n=NT),
                    out_offset=None,
                    in_=cen_d[:],
                    in_offset=bass.IndirectOffsetOnAxis(ap=off_ap, axis=0),
                ).then_inc(s_g[-1], 16)
            )
        else:
            for t in range(NT):
                if dram_off:
                    off_ap = lab_d[:].rearrange("(p n) o -> p (n o)", n=NT)[
                        :, t : t + 1
                    ]
                else:
                    off_ap = idx[:, t : t + 1]
                pl(
                    nc.gpsimd.indirect_dma_start(
                        out=c_all[:, t * D : (t + 1) * D],
                        out_offset=None,
                        in_=cen_d[:],
                        in_offset=bass.IndirectOffsetOnAxis(ap=off_ap, axis=0),
                    ).then_inc(s_g[t], 16)
                )
        if warmup:
            # Prove the warmup queue drained (nobody else waits on it and
            # the kernel-tail drain only covers SP queues).
            pl(nc.gpsimd.wait_ge(s_w, 16))
        if hoist_pl:
            hoisted.extend(pl_insts)

        # --- DVE: per-tile subtract + fused square/row-sum ------------
        # scalar_tensor_tensor: out = (in0 op0 scalar) op1 in1, accum_out
        # = sum(out).  With op0=bypass/op1=mult and in0=in1=dif this is
        # sum(dif^2) per partition row in ONE instruction.
        # RAW hazards between back-to-back dependent DVE ops are real
        # (the pipe flush only covers output hazards), so each tile's
        # stt chains behind its sub through the s_d self-semaphore.
        nc.vector.sem_clear(s_d)
        nc.vector.sem_clear(s_v)
        nc.vector.wait_ge(s_x, 16)
        for t in range(NT):
            cols = slice(t * D, (t + 1) * D)
            nc.vector.wait_ge(s_g[-1] if single_gather else s_g[t], 16)
            nc.vector.tensor_tensor(
                out=dif[:, cols],
                in0=x_all[:, cols],
                in1=c_all[:, cols],
                op=mybir.AluOpType.subtract,
            ).then_inc(s_d, 1)
            nc.vector.wait_ge(s_d, t + 1)
            nc.vector.scalar_tensor_tensor(
                out=sq[:, cols],
                in0=dif[:, cols],
                scalar=0.0,
                in1=dif[:, cols],
                op0=mybir.AluOpType.bypass,
                op1=mybir.AluOpType.mult,
                accum_out=s_all[:, t : t + 1],
            ).then_inc(s_v, 1)

        # --- SP tail: results out, then one cheap drain ---------------
        nc.sync.sem_clear(s_out)
        if split_out:
            # Tile 0's store overlaps tile 1's DVE math; only tile 1's
            # small store remains on the tail.
            for t in range(NT):
                nc.sync.wait_ge(s_v, t + 1)
                nc.sync.dma_start(
                    out=out_d[t : t + 1, :].rearrange("n p -> p n"),
                    in_=s_all[:, t : t + 1],
                ).then_inc(s_out, 16)
        else:
            nc.sync.wait_ge(s_v, NT)
            nc.sync.dma_start(
                out=out_d[:].rearrange("(p n) o -> p (n o)", n=NT), in_=s_all[:]
            ).then_inc(s_out, 16)
        nc.sync.drain()

    if hoist:
        block = nc.m.functions[0].blocks[0]
        insts = block.instructions
        moved = [i for i in insts if any(i is h for h in hoisted)]
        rest = [i for i in insts if not any(i is h for h in hoisted)]
        # Keep the leading dummy Call first, then our SP DMAs, then all
        # framework preamble + kernel body.
        assert type(rest[0]).__name__ == "InstCall", type(rest[0]).__name__
        block.instructions[:] = rest[:1] + moved + rest[1:]

    return nc


def _build_raw():
    """Original hand-synchronized raw-Bass kernel (baseline fallback)."""
    from contextlib import ExitStack

    import concourse.bass as bass
    import concourse.mybir as mybir

    f32 = mybir.dt.float32
    i32 = mybir.dt.int32
    NT = TILES_PER_CORE
    D = FEAT_DIM

    nc = bass.Bass()
    x_d = nc.dram_tensor("x", [ROWS_PER_CORE, D], f32, kind="ExternalInput")
    lab_d = nc.dram_tensor("labels", [ROWS_PER_CORE, 1], i32, kind="ExternalInput")
    cen_d = nc.dram_tensor("centers", [NUM_CLASSES, D], f32, kind="ExternalInput")
    out_d = nc.dram_tensor("dists", [ROWS_PER_CORE, 1], f32, kind="ExternalOutput")

    with ExitStack() as ctx:
        x_all = ctx.enter_context(nc.sbuf_tensor([P, NT * D], f32))
        idx = ctx.enter_context(nc.sbuf_tensor([P, NT], i32))
        c_all = ctx.enter_context(nc.sbuf_tensor([P, NT * D], f32))
        dif = ctx.enter_context(nc.sbuf_tensor([P, NT * D], f32))
        sq = ctx.enter_context(nc.sbuf_tensor([P, NT * D], f32))
        s_all = ctx.enter_context(nc.sbuf_tensor([P, NT], f32))
        s_lab = ctx.enter_context(nc.semaphore("s_lab"))
        s_x = ctx.enter_context(nc.semaphore("s_x"))
        s_g = ctx.enter_context(nc.semaphore("s_g"))
        s_v = ctx.enter_context(nc.semaphore("s_v"))
        s_out = ctx.enter_context(nc.semaphore("s_out"))
        s_d = ctx.enter_context(nc.semaphore("s_d"))

        for s in (s_x, s_g, s_v, s_out, s_d):
            nc.gpsimd.sem_clear(s)
        nc.sync.sem_clear(s_lab)
        nc.sync.dma_start(
            out=idx[:], in_=lab_d[:].rearrange("(p n) o -> p (n o)", n=NT)
        ).then_inc(s_lab, 16)
        nc._nrt_pseudo_barrier()

        nc.sync.dma_start(
            out=x_all[:].rearrange("p (n d) -> p n d", n=NT),
            in_=x_d[:].rearrange("(p n) d -> p n d", n=NT),
        ).then_inc(s_x, 16)

        nc.gpsimd.wait_ge(s_lab, 16)
        for t in range(NT):
            nc.gpsimd.indirect_dma_start(
                out=c_all[:, t * D : (t + 1) * D],
                out_offset=None,
                in_=cen_d[:],
                in_offset=bass.IndirectOffsetOnAxis(ap=idx[:, t : t + 1], axis=0),
            ).then_inc(s_g, 16)

        nc.vector.wait_ge(s_x, 16)
        nc.vector.wait_ge(s_g, 16 * NT)
        nc.vector.tensor_tensor(
            out=dif[:],
            in0=x_all[:],
            in1=c_all[:],
            op=mybir.AluOpType.subtract,
        ).then_inc(s_d, 1)
        nc.vector.wait_ge(s_d, 1)
        nc.vector.tensor_tensor(
            out=sq[:], in0=dif[:], in1=dif[:], op=mybir.AluOpType.mult
        ).then_inc(s_d, 1)
        nc.vector.wait_ge(s_d, 2)
        nc.vector.tensor_reduce(
            out=s_all[:],
            in_=sq[:].rearrange("p (n d) -> p n d", n=NT),
            axis=mybir.AxisListType.X,
            op=mybir.AluOpType.add,
        ).then_inc(s_v, 1)

        nc.sync.wait_ge(s_v, 1)
        nc.sync.dma_start(
            out=out_d[:].rearrange("(p n) o -> p (n o)", n=NT), in_=s_all[:]
        ).then_inc(s_out, 16)
        nc.sync.drain()

    return nc


def _build(impl):
    if impl == "raw":
        return _build_raw()
    if impl == "v2":
        return _build_v2(hoist=False, warmup=False, dram_off=False)
    if impl == "v2h":
        return _build_v2(hoist=True, warmup=False, dram_off=False)
    if impl == "v2w":
        return _build_v2(hoist=False, warmup=True, dram_off=False)
    if impl == "v2hw":
        return _build_v2(hoist=True, warmup=True, dram_off=False)
    if impl == "v2hp":
        # warmup + gathers hoisted ahead of the Pool preamble too
        return _build_v2(hoist=True, warmup=True, dram_off=False, hoist_pl=True)
    if impl == "v2hp1":
        return _build_v2(
            hoist=True, warmup=True, dram_off=False, hoist_pl=True,
            single_gather=True,
        )
    if impl == "v2h1":
        return _build_v2(
            hoist=True, warmup=False, dram_off=False, single_gather=True
        )
    if impl == "dram":
        return _build_v2(hoist=True, warmup=False, dram_off=True, hoist_pl=True)
    if impl == "v2hs":
        return _build_v2(hoist=True, warmup=False, split_out=True)
    if impl == "v2hsn":
        return _build_v2(
            hoist=True, warmup=False, split_out=True, pregather_nop=True
        )
    if impl == "v3":
        return _build_v2(
            hoist=True, warmup=False, split_out=True, x_on_act=True,
            lab_single_packet=True,
        )
    if impl == "v3sp":
        return _build_v2(
            hoist=True, warmup=False, split_out=True, lab_single_packet=True
        )
    if impl == "v3xa":
        return _build_v2(
            hoist=True, warmup=False, split_out=True, x_on_act=True
        )
    if impl == "v4":
        return _build_v2(
            hoist=True, warmup=False, split_out=True, x_on_act=True,
            lab_single_packet=True, x_wait_lab=True,
        )
    raise ValueError(impl)


def kernel(x, labels, centers):
    from concourse.bass_utils import run_bass_kernel_spmd

    x = np.ascontiguousarray(np.asarray(x, dtype=np.float32))
    centers = np.ascontiguousarray(np.asarray(centers, dtype=np.float32))
    labels = np.ascontiguousarray(
        np.asarray(labels).astype(np.int32).reshape(BATCH, 1)
    )

    impl = os.environ.get("CENTERLOSS_IMPL", "v2hw")
    if ("nc", impl) not in _CACHE:
        _CACHE[("nc", impl)] = _build(impl)
    nc = _CACHE[("nc", impl)]

    core_ids = list(range(N_CORES))
    in_maps = [
        {
            "x": x[k * ROWS_PER_CORE : (k + 1) * ROWS_PER_CORE],
            "labels": labels[k * ROWS_PER_CORE : (k + 1) * ROWS_PER_CORE],
            "centers": centers,
        }
        for k in core_ids
    ]

    res = run_bass_kernel_spmd(nc, in_maps, core_ids)
    _CACHE["last_results"] = res

    def _rows(d):
        d = np.asarray(d)
        if d.shape == (TILES_PER_CORE, P):  # tile-major -> row-ordered
            return d.T.reshape(-1)
        return d.reshape(-1)

    dists = np.concatenate([_rows(res.results[k]["dists"]) for k in core_ids])
    # Reference clamps after masking: the label entry per row is clipped to
    # [1e-12, 1e12], and the B*(C-1) masked zeros each become 1e-12.
    dists = np.clip(dists, 1e-12, 1e12)
    total = dists.sum(dtype=np.float64) + BATCH * (NUM_CLASSES - 1) * 1e-12
    return np.float32(total / BATCH)
